# revision 17
# baseline (speedup 1.0000x reference)
"""Trainium2 Bass kernel for nn_Attention_40037685133427.

FiLM-conditioned LayerNorm + 16-head self-attention (B=2, N=2048, D=1024),
tensor-parallel over 8 NeuronCores: core c owns heads {2c, 2c+1}.

v3 (from 554us baseline -> 400us v2):
  - FiLM affine folded on HOST into per-batch QKV weights + per-column
    correction constants (no condW DMA, no on-device film stage).
  - LN stats stay on-chip: per-isl sums accumulate at PSUM partitions
    {0,32,64,96}, vectorized Newton rsqrt runs per isl-PAIR (so PE work
    overlaps the serial DVE chain), PE broadcasts read stat rows in place.
  - QKV psum evacuated raw by ACT; LN correction applied in-place on SBUF
    by DVE once U/MU broadcasts land (decouples PSUM from stats latency).
  - V tiles transposed by the DMA xbar (sync+scalar HWDGE queues).
  - Softmax exp alternates per key-tile between ACT (table exp) and DVE
    (Schraudolph: int16(23.083*S + 16248.6) bitcast bf16 ~ bf16(exp(S/8))).
  - attn@V / denominator are 2x column-tiled M=64 pairs accumulating over
    start=False into zero-matmul-initialized banks.
  - Attention software pipeline: S/exp run 5 key-tiles ahead of attn@V;
    projection of slice i is emitted inside slice i+1 so the reciprocal+
    normalize chain never idles the PE; 3-deep [128,1024] S-tile ring.
Host sums the 8 partial y^T outputs (row-split Wo => partial sums).
"""

import sys

sys.path.insert(0, "/opt/trn_rl_repo")

import math
import numpy as np
import ml_dtypes

import concourse.bass as bass
from concourse import bacc
import concourse.tile as tile
from concourse import mybir
from concourse.bass_utils import run_bass_kernel_spmd

f32 = mybir.dt.float32
bf16 = mybir.dt.bfloat16
i16 = mybir.dt.int16
AF = mybir.ActivationFunctionType
ALU = mybir.AluOpType

B, N, DIM = 2, 2048, 1024
HEADS, DH = 16, 64
TOK = B * N            # 4096 tokens, batch-major
KT = DIM // 128        # 8 k-tiles over the model dim
NSL = 8                # 8 token slices of 512
JT = N // 128          # 16 key tiles per batch
NCORES = 8

A_SCH = (128.0 / math.log(2.0)) * (DH ** -0.5)   # 23.0831...
B_SCH = 16256.0 - 7.4
AV_LAG = 5


def build_program():
    nc = bacc.Bacc("TRN2", target_bir_lowering=False, debug=False)

    xT = nc.dram_tensor("xT", [DIM, TOK], bf16, kind="ExternalInput").ap()
    wqkv = nc.dram_tensor("wqkv", [DIM, 2 * 384], bf16, kind="ExternalInput").ap()
    wcorr = nc.dram_tensor("wcorr", [128, 12], f32, kind="ExternalInput").ap()
    wo = nc.dram_tensor("wo", [128, DIM], bf16, kind="ExternalInput").ap()

    yT_out = nc.dram_tensor("yT", [DIM, TOK], bf16, kind="ExternalOutput").ap()

    with tile.TileContext(nc) as tc:
        with (
            tc.tile_pool(name="const", bufs=1) as const,
            tc.tile_pool(name="persist", bufs=1) as persist,
            tc.tile_pool(name="work", bufs=2) as work,
            tc.tile_pool(name="ps", bufs=8, space="PSUM") as ps,
        ):
            def st2t():
                # S tiles / QKV psum / broadcasts: 2-deep [128,1024] ring
                return ps.tile([128, 1024], f32, tag="st2", bufs=2, name="st2t")

            def podt():
                # attn@V + den accumulators (4 pinned per slice), proj outputs,
                # LN stats banks, all [128,512]
                return ps.tile([128, 512], f32, tag="pod", bufs=4, name="podt")

            # ---------------- constants / weights ----------------
            ones_col = const.tile([128, 1], bf16)
            nc.vector.memset(ones_col[:], 1.0)
            ones_b = const.tile([128, 128], bf16)
            nc.vector.memset(ones_b[:], 1.0)
            ones64 = const.tile([128, 64], bf16)
            nc.vector.memset(ones64[:], 1.0)
            zeros64 = const.tile([128, 64], bf16)
            nc.vector.memset(zeros64[:], 0.0)
            warm = const.tile([1, 16], f32)
            nc.vector.memset(warm[:], 0.0)
            nc.scalar.activation(warm[:], warm[:], AF.Exp)  # ACT exp table warmup

            wq_sb = []
            for kt in range(KT):
                wg = persist.tile([128, 768], bf16, tag="wg", bufs=KT)
                nc.gpsimd.dma_start(wg[:], wqkv[kt * 128:(kt + 1) * 128, :])
                wq_sb.append(wg)
            wo_sb = persist.tile([128, DIM], bf16, tag="wo")
            nc.gpsimd.dma_start(wo_sb[:], wo)
            wc = const.tile([128, 12], f32)
            nc.gpsimd.dma_start(wc[:], wcorr)

            # x loads: [128, 2048] per (group, kt); kt 0-3 sync, 4-7 scalar
            xg = [[None] * KT for _ in range(2)]
            for g in range(2):
                gsl = slice(g * 2048, (g + 1) * 2048)
                for kt in range(KT):
                    xb = persist.tile([128, 2048], bf16, tag="xg", bufs=16,
                                      name=f"x{g}_{kt}")
                    eng = nc.sync if kt < 4 else nc.scalar
                    eng.dma_start(xb[:], xT[kt * 128:(kt + 1) * 128, gsl])
                    xg[g][kt] = xb

            # persistent SBUF state
            q2T = persist.tile([128, TOK], bf16, tag="q2T")
            k2T = persist.tile([128, TOK], bf16, tag="k2T")
            V2 = [None] * (B * JT)
            U_sb = [None] * NSL
            MU_sb = [None] * NSL
            vraw = [None] * NSL

            yq = [nc.sync, nc.gpsimd]   # output dma queues, round robin
            tq = [nc.sync, nc.scalar]   # transpose queues (HWDGE only)

            def qkv_isl(g, r, sA, sB):
                """stats + raw QKV for isl = g*4 + r"""
                b = g
                isl = g * 4 + r
                sl_g = slice(r * 512, (r + 1) * 512)
                sl = slice(isl * 512, (isl + 1) * 512)
                xsq = []
                for kt in range(KT):
                    xq = work.tile([128, 512], bf16, tag="xsq", bufs=3)
                    nc.gpsimd.tensor_tensor(xq[:], xg[g][kt][:, sl_g],
                                            xg[g][kt][:, sl_g], op=ALU.mult)
                    xsq.append(xq)
                p = 32 * r
                for kt in range(KT):
                    nc.tensor.matmul(sA[p:p + 1, :], ones_col[:],
                                     xg[g][kt][:, sl_g],
                                     start=(kt == 0), stop=(kt == KT - 1),
                                     tile_position=(0, p))
                    nc.tensor.matmul(sB[p:p + 1, :], ones_col[:], xsq[kt][:],
                                     start=(kt == 0), stop=(kt == KT - 1),
                                     tile_position=(0, p))
                for pj in (2, 1, 0):
                    pq = st2t()
                    for kt in range(KT):
                        nc.tensor.matmul(
                            pq[:, 0:512],
                            wq_sb[kt][:, b * 384 + pj * 128:
                                      b * 384 + (pj + 1) * 128],
                            xg[g][kt][:, sl_g],
                            start=(kt == 0), stop=(kt == KT - 1))
                    if pj == 2:
                        vr = persist.tile([128, 512], bf16, tag="vraw",
                                          bufs=4, name=f"vraw{isl}")
                        vraw[isl] = vr
                        nc.scalar.copy(vr[:], pq[:, 0:512])
                    elif pj == 1:
                        nc.scalar.copy(k2T[:, sl], pq[:, 0:512])
                    else:
                        nc.scalar.copy(q2T[:, sl], pq[:, 0:512])

            def newton_pair(g, rr, sA, sB):
                """rsqrt stats + broadcasts + corrections + V transposes for
                isls g*4+rr, g*4+rr+1 (stat rows 32*rr, 32*(rr+1))."""
                b = g
                tsum = work.tile([128, 512], f32, tag="tsum", bufs=2)
                tsq = work.tile([128, 512], f32, tag="tsq", bufs=2)
                nc.scalar.copy(tsum[:], sA[:])
                nc.scalar.copy(tsq[:], sB[:])
                mean = work.tile([128, 512], f32, tag="nmean", bufs=2)
                nc.vector.tensor_scalar(mean[:], tsum[:], 1.0 / DIM, None, ALU.mult)
                var = work.tile([128, 512], f32, tag="nvar", bufs=2)
                nc.vector.tensor_scalar(var[:], tsq[:], 1.0 / DIM, 1e-5,
                                        ALU.mult, ALU.add)
                msq = work.tile([128, 512], f32, tag="nmsq", bufs=2)
                nc.vector.tensor_tensor(msq[:], mean[:], mean[:], op=ALU.mult)
                nc.vector.tensor_tensor(var[:], var[:], msq[:], op=ALU.subtract)
                u = work.tile([128, 512], f32, tag="nu", bufs=2)
                nc.vector.tensor_scalar(u[:], var[:], -0.5, 1.5, ALU.mult, ALU.add)
                nwt = work.tile([128, 512], f32, tag="nwt", bufs=2)
                for _ in range(2):
                    nc.vector.tensor_tensor(nwt[:], u[:], u[:], op=ALU.mult)
                    nc.vector.tensor_tensor(nwt[:], nwt[:], var[:], op=ALU.mult)
                    nc.vector.tensor_scalar(nwt[:], nwt[:], -0.5, 1.5,
                                            ALU.mult, ALU.add)
                    nc.vector.tensor_tensor(u[:], u[:], nwt[:], op=ALU.mult)
                mu = work.tile([128, 512], f32, tag="nmu", bufs=2)
                nc.vector.tensor_tensor(mu[:], mean[:], u[:], op=ALU.mult)
                ub = work.tile([128, 512], bf16, tag="nub", bufs=2)
                mub = work.tile([128, 512], bf16, tag="nmub", bufs=2)
                nc.vector.tensor_copy(ub[:], u[:])
                nc.vector.tensor_copy(mub[:], mu[:])

                for r in (rr, rr + 1):
                    isl = g * 4 + r
                    sl = slice(isl * 512, (isl + 1) * 512)
                    p = 32 * r
                    pU = st2t()
                    nc.tensor.matmul(pU[:, 0:512], ones_b[p:p + 1, :],
                                     ub[p:p + 1, :],
                                     start=True, stop=True, tile_position=(p, 0))
                    nc.tensor.matmul(pU[:, 512:1024], ones_b[p:p + 1, :],
                                     mub[p:p + 1, :],
                                     start=True, stop=True, tile_position=(p, 0))
                    usb = persist.tile([128, 512], bf16, tag="Usb", bufs=NSL,
                                       name=f"U{isl}")
                    nc.scalar.copy(usb[:], pU[:, 0:512])
                    U_sb[isl] = usb
                    musb = persist.tile([128, 512], bf16, tag="MUsb", bufs=NSL,
                                        name=f"MU{isl}")
                    nc.scalar.copy(musb[:], pU[:, 512:1024])
                    MU_sb[isl] = musb
                    for pj, dest in ((0, q2T[:, sl]), (1, k2T[:, sl]),
                                     (2, vraw[isl][:])):
                        w2 = work.tile([128, 512], bf16, tag="w2", bufs=3)
                        nc.gpsimd.tensor_scalar(w2[:], musb[:],
                                                wc[:, b * 6 + pj:b * 6 + pj + 1],
                                                wc[:, b * 6 + 3 + pj:b * 6 + 4 + pj],
                                                ALU.mult, ALU.add)
                        nc.vector.tensor_tensor(dest, dest, usb[:], op=ALU.mult)
                        nc.vector.tensor_tensor(dest, dest, w2[:], op=ALU.add)
                    for q4 in range(4):
                        gj = b * JT + r * 4 + q4
                        v2 = persist.tile([128, 128], bf16, tag="V2", bufs=B * JT,
                                          name=f"V2_{gj}")
                        tq[gj % 2].dma_start_transpose(
                            v2[:], vraw[isl][:, q4 * 128:(q4 + 1) * 128])
                        V2[gj] = v2

            def qkv_group(g):
                sA = podt()   # LN sums,    isl r at partition 32r
                sB = podt()   # LN sumsqs,  isl r at partition 32r
                qkv_isl(g, 0, sA, sB)
                qkv_isl(g, 1, sA, sB)
                newton_pair(g, 0, sA, sB)
                qkv_isl(g, 2, sA, sB)
                qkv_isl(g, 3, sA, sB)
                newton_pair(g, 2, sA, sB)

            def attn_slice(b, islq, prev_norm, prev_proj):
                isl = b * 4 + islq
                qsl = slice(b * N + islq * 512, b * N + (islq + 1) * 512)
                acc = [None] * 4          # po_A, po_B, dn_A, dn_B
                pt2s = [None] * JT

                def avden(jt):
                    # 8 matmuls, all (64,64) tiles: 4-way concurrent quads.
                    # keylo rows -> *_A banks, keyhi rows -> *_B banks.
                    lst = (jt == JT - 1)
                    gj = b * JT + jt
                    pt2 = pt2s[jt]
                    po_a, po_b, dn_a, dn_b = acc
                    for h in range(2):
                        csl = slice(h * 512, (h + 1) * 512)
                        dsl = slice(h * 64, (h + 1) * 64)
                        nc.tensor.matmul(po_a[dsl, :], V2[gj][0:64, dsl],
                                         pt2[0:64, csl], start=False, stop=lst,
                                         tile_position=(0, h * 64))
                        nc.tensor.matmul(po_b[dsl, :], V2[gj][64:128, dsl],
                                         pt2[64:128, csl], start=False, stop=lst,
                                         tile_position=(64, h * 64))
                        nc.tensor.matmul(dn_a[dsl, :], ones64[0:64, :],
                                         pt2[0:64, csl], start=False, stop=lst,
                                         tile_position=(0, h * 64))
                        nc.tensor.matmul(dn_b[dsl, :], ones64[64:128, :],
                                         pt2[64:128, csl], start=False, stop=lst,
                                         tile_position=(64, h * 64))

                for jt in range(JT):
                    klo = slice(b * N + jt * 128, b * N + jt * 128 + 64)
                    khi = slice(b * N + jt * 128 + 64, b * N + (jt + 1) * 128)
                    st = st2t()
                    for h in range(2):
                        rsl = slice(h * 64, (h + 1) * 64)
                        csl = slice(h * 512, (h + 1) * 512)
                        nc.tensor.matmul(st[0:64, csl], k2T[rsl, klo],
                                         q2T[rsl, qsl], start=True, stop=True,
                                         tile_position=(h * 64, 0))
                        nc.tensor.matmul(st[64:128, csl], k2T[rsl, khi],
                                         q2T[rsl, qsl], start=True, stop=True,
                                         tile_position=(h * 64, 64))
                    pt2 = work.tile([128, 1024], bf16, tag="pt2", bufs=AV_LAG + 2)
                    if jt % 2 == 1 and jt < 14:    # 7 on DVE, 9 on ACT
                        nc.vector.tensor_scalar(pt2[:].bitcast(i16), st[:],
                                                A_SCH, B_SCH, ALU.mult, ALU.add)
                    else:
                        nc.scalar.activation(pt2[:], st[:], AF.Exp, scale=DH ** -0.5)
                    pt2s[jt] = pt2
                    if jt == 2 and prev_norm is not None:
                        prev_norm()
                    if jt == AV_LAG - 1:
                        # zero-init the 4 accumulator banks (robust under either
                        # has_written-clear semantics; attn matmuls accumulate)
                        acc = [podt() for _ in range(4)]
                        for pz in acc:
                            nc.tensor.matmul(pz[0:64, :], zeros64[0:64, :],
                                             q2T[0:64, qsl], start=True,
                                             stop=True, tile_position=(0, 0))
                            nc.tensor.matmul(pz[64:128, :], zeros64[0:64, :],
                                             q2T[0:64, qsl], start=True,
                                             stop=True, tile_position=(0, 64))
                    if jt >= AV_LAG:
                        avden(jt - AV_LAG)
                for jt in range(JT - AV_LAG, JT):
                    avden(jt)
                po_a, po_b, dn_a, dn_b = acc
                o2t = work.tile([128, 512], bf16, tag="o2t", bufs=2)

                def norm():
                    # ordered so the psum banks free first, recip last
                    pob_sb = work.tile([128, 512], bf16, tag="pob", bufs=2)
                    nc.scalar.copy(pob_sb[:], po_b[:])
                    dnb_sb = work.tile([128, 512], f32, tag="dnb", bufs=2)
                    nc.scalar.copy(dnb_sb[:], dn_b[:])
                    osum = work.tile([128, 512], bf16, tag="osum", bufs=2)
                    nc.vector.tensor_tensor(osum[:], po_a[:], pob_sb[:], op=ALU.add)
                    dsum = work.tile([128, 512], f32, tag="dsum", bufs=2)
                    nc.vector.tensor_tensor(dsum[:], dn_a[:], dnb_sb[:], op=ALU.add)
                    rb = work.tile([128, 512], f32, tag="rb", bufs=2)
                    nc.vector.reciprocal(rb[:], dsum[:])
                    nc.gpsimd.tensor_tensor(o2t[:], osum[:], rb[:], op=ALU.mult)

                def proj():
                    for half in range(4):
                        py = st2t()
                        for k in range(2):
                            ncx = half * 2 + k
                            nc.tensor.matmul(py[:, 512 * k:512 * (k + 1)],
                                             wo_sb[:, ncx * 128:(ncx + 1) * 128],
                                             o2t[:], start=True, stop=True)
                        yb = work.tile([128, 1024], bf16, tag="yb", bufs=3)
                        if half % 4 != 1:
                            nc.scalar.copy(yb[:], py[:])
                        else:
                            nc.vector.tensor_copy(yb[:], py[:])
                        for k in range(2):
                            ncx = half * 2 + k
                            yq[(half + k) % 2].dma_start(
                                yT_out[ncx * 128:(ncx + 1) * 128, qsl],
                                yb[:, 512 * k:512 * (k + 1)])
                if prev_proj is not None:
                    prev_proj()
                return norm, proj

            qkv_group(0)
            pn = pp = None
            for islq in range(4):
                pn, pp = attn_slice(0, islq, pn, pp)
            pn()
            pp()
            qkv_group(1)
            pn = pp = None
            for islq in range(4):
                pn, pp = attn_slice(1, islq, pn, pp)
            pn()
            pp()

    nc.compile()
    return nc


_NC_CACHE = None


def _get_nc():
    global _NC_CACHE
    if _NC_CACHE is None:
        _NC_CACHE = build_program()
    return _NC_CACHE


def make_in_maps(x, conditioning_embeddings, gamma, cond_W, cond_b, Wq, Wkv, Wo):
    x = np.asarray(x, np.float32)
    ce = np.asarray(conditioning_embeddings, np.float32)
    gamma = np.asarray(gamma, np.float32)
    cond_W = np.asarray(cond_W, np.float32)
    cond_b = np.asarray(cond_b, np.float32)
    Wq = np.asarray(Wq, np.float32)
    Wkv = np.asarray(Wkv, np.float32)
    Wo = np.asarray(Wo, np.float32)

    bf = ml_dtypes.bfloat16
    xT = np.ascontiguousarray(x.reshape(TOK, DIM).T).astype(bf)

    # FiLM on host: silu -> linear -> (scale, shift); fold into QKV weights
    cond = (ce / (1.0 + np.exp(-ce))) @ cond_W + cond_b          # [B, 2D]
    scale, shift = cond[:, :DIM], cond[:, DIM:]                   # [B, D]
    gpf = (scale + 1.0) * gamma                                   # [B, D]

    in_maps = []
    for c in range(NCORES):
        cs = slice(128 * c, 128 * (c + 1))
        Wc = np.concatenate(
            [Wq[:, cs], Wkv[:, cs], Wkv[:, 1024 + 128 * c:1024 + 128 * (c + 1)]],
            axis=1)                                               # [D, 384]
        wq_b = []
        for b in range(B):
            wgb = (Wc * gpf[b][:, None]).astype(bf)               # [D, 384] bf16
            wq_b.append(wgb)
        # wcorr layout: col b*6+p = wgs_neg[b] slice p; col b*6+3+p = wbs[b] slice p
        wcorr = np.zeros((128, 12), np.float32)
        for b in range(B):
            wgs_neg = -wq_b[b].astype(np.float32).sum(axis=0)
            wbs = shift[b] @ Wc
            for p in range(3):
                wcorr[:, b * 6 + p] = wgs_neg[p * 128:(p + 1) * 128]
                wcorr[:, b * 6 + 3 + p] = wbs[p * 128:(p + 1) * 128]
        in_maps.append({
            "xT": xT,
            "wqkv": np.ascontiguousarray(np.concatenate(wq_b, axis=1)),
            "wcorr": wcorr,
            "wo": np.ascontiguousarray(Wo[cs, :]).astype(bf),
        })
    return in_maps


def kernel(**inputs) -> np.ndarray:
    nc = _get_nc()
    in_maps = make_in_maps(**inputs)
    res = run_bass_kernel_spmd(nc, in_maps, core_ids=list(range(NCORES)))
    acc = np.zeros((DIM, TOK), np.float32)
    for core in res.results:
        acc += np.asarray(core["yT"]).astype(np.float32)
    return np.ascontiguousarray(acc.T).reshape(B, N, DIM)


# revision 21
# speedup vs baseline: 1.0737x; 1.0737x over previous
"""Trainium2 Bass kernel for nn_Attention_40037685133427.

FiLM-conditioned LayerNorm + 16-head self-attention (B=2, N=2048, D=1024),
tensor-parallel over 8 NeuronCores: core c owns heads {2c, 2c+1}.

v3 (from 554us baseline -> 400us v2):
  - FiLM affine folded on HOST into per-batch QKV weights + per-column
    correction constants (no condW DMA, no on-device film stage).
  - LN stats stay on-chip: per-isl sums accumulate at PSUM partitions
    {0,32,64,96}, vectorized Newton rsqrt runs per isl-PAIR (so PE work
    overlaps the serial DVE chain), PE broadcasts read stat rows in place.
  - QKV psum evacuated raw by ACT; LN correction applied in-place on SBUF
    by DVE once U/MU broadcasts land (decouples PSUM from stats latency).
  - V tiles transposed by the DMA xbar (sync+scalar HWDGE queues).
  - Softmax exp alternates per key-tile between ACT (table exp) and DVE
    (Schraudolph: int16(23.083*S + 16248.6) bitcast bf16 ~ bf16(exp(S/8))).
  - attn@V / denominator are 2x column-tiled M=64 pairs accumulating over
    start=False into zero-matmul-initialized banks.
  - Attention software pipeline: S/exp run 5 key-tiles ahead of attn@V;
    projection of slice i is emitted inside slice i+1 so the reciprocal+
    normalize chain never idles the PE; 3-deep [128,1024] S-tile ring.
Host sums the 8 partial y^T outputs (row-split Wo => partial sums).
"""

import sys

sys.path.insert(0, "/opt/trn_rl_repo")

import math
import numpy as np
import ml_dtypes

import concourse.bass as bass
from concourse import bacc
import concourse.tile as tile
from concourse import mybir
from concourse.bass_utils import run_bass_kernel_spmd

f32 = mybir.dt.float32
bf16 = mybir.dt.bfloat16
i16 = mybir.dt.int16
AF = mybir.ActivationFunctionType
ALU = mybir.AluOpType

B, N, DIM = 2, 2048, 1024
HEADS, DH = 16, 64
TOK = B * N            # 4096 tokens, batch-major
KT = DIM // 128        # 8 k-tiles over the model dim
NSL = 8                # 8 token slices of 512
JT = N // 128          # 16 key tiles per batch
NCORES = 8

A_SCH = (128.0 / math.log(2.0)) * (DH ** -0.5)   # 23.0831...
B_SCH = 16256.0 - 7.4
AV_LAG = 5


def build_program():
    nc = bacc.Bacc("TRN2", target_bir_lowering=False, debug=False)

    xT = nc.dram_tensor("xT", [DIM, TOK], bf16, kind="ExternalInput").ap()
    wqkv = nc.dram_tensor("wqkv", [DIM, 2 * 384], bf16, kind="ExternalInput").ap()
    wcorr = nc.dram_tensor("wcorr", [128, 12], f32, kind="ExternalInput").ap()
    wo = nc.dram_tensor("wo", [128, DIM], bf16, kind="ExternalInput").ap()

    yT_out = nc.dram_tensor("yT", [DIM, TOK], bf16, kind="ExternalOutput").ap()

    with tile.TileContext(nc) as tc:
        with (
            tc.tile_pool(name="const", bufs=1) as const,
            tc.tile_pool(name="persist", bufs=1) as persist,
            tc.tile_pool(name="work", bufs=2) as work,
            tc.tile_pool(name="ps", bufs=8, space="PSUM") as ps,
        ):
            def st2t():
                # S tiles / QKV psum / broadcasts: 2-deep [128,1024] ring
                return ps.tile([128, 1024], f32, tag="st2", bufs=2, name="st2t")

            def podt():
                # attn@V + den accumulators (4 pinned per slice), proj outputs,
                # LN stats banks, all [128,512]
                return ps.tile([128, 512], f32, tag="pod", bufs=4, name="podt")

            # ---------------- constants / weights ----------------
            ones_col = const.tile([128, 1], bf16)
            nc.vector.memset(ones_col[:], 1.0)
            ones_b = const.tile([128, 128], bf16)
            nc.vector.memset(ones_b[:], 1.0)
            ones64 = const.tile([128, 64], bf16)
            nc.vector.memset(ones64[:], 1.0)
            zeros64 = const.tile([128, 64], bf16)
            nc.vector.memset(zeros64[:], 0.0)
            warm = const.tile([1, 16], f32)
            nc.vector.memset(warm[:], 0.0)
            nc.scalar.activation(warm[:], warm[:], AF.Exp)  # ACT exp table warmup

            wq_sb = []
            for kt in range(KT):
                wg = persist.tile([128, 768], bf16, tag="wg", bufs=KT)
                nc.gpsimd.dma_start(wg[:], wqkv[kt * 128:(kt + 1) * 128, :])
                wq_sb.append(wg)
            wo_sb = persist.tile([128, DIM], bf16, tag="wo")
            nc.gpsimd.dma_start(wo_sb[:], wo)
            wc = const.tile([128, 12], f32)
            nc.gpsimd.dma_start(wc[:], wcorr)

            # x loads: [128, 2048] per (group, kt); kt 0-3 sync, 4-7 scalar
            xg = [[None] * KT for _ in range(2)]
            for g in range(2):
                gsl = slice(g * 2048, (g + 1) * 2048)
                for kt in range(KT):
                    xb = persist.tile([128, 2048], bf16, tag="xg", bufs=16,
                                      name=f"x{g}_{kt}")
                    eng = nc.sync if kt < 4 else nc.scalar
                    eng.dma_start(xb[:], xT[kt * 128:(kt + 1) * 128, gsl])
                    xg[g][kt] = xb

            # persistent SBUF state
            q2T = persist.tile([128, TOK], bf16, tag="q2T")
            k2T = persist.tile([128, TOK], bf16, tag="k2T")
            V2 = [None] * (B * JT)
            U_sb = [None] * NSL
            MU_sb = [None] * NSL
            vraw = [None] * NSL

            yq = [nc.sync, nc.gpsimd]   # output dma queues, round robin
            tq = [nc.sync, nc.scalar]   # transpose queues (HWDGE only)

            def qkv_isl(g, r, sA, sB):
                """stats + raw QKV for isl = g*4 + r"""
                b = g
                isl = g * 4 + r
                sl_g = slice(r * 512, (r + 1) * 512)
                sl = slice(isl * 512, (isl + 1) * 512)
                xsq = []
                for kt in range(KT):
                    xq = work.tile([128, 512], bf16, tag="xsq", bufs=3)
                    nc.vector.tensor_tensor(xq[:], xg[g][kt][:, sl_g],
                                            xg[g][kt][:, sl_g], op=ALU.mult)
                    xsq.append(xq)
                p = 32 * r
                for kt in range(KT):
                    nc.tensor.matmul(sA[p:p + 1, :], ones_col[:],
                                     xg[g][kt][:, sl_g],
                                     start=(kt == 0), stop=(kt == KT - 1),
                                     tile_position=(0, p))
                    nc.tensor.matmul(sB[p:p + 1, :], ones_col[:], xsq[kt][:],
                                     start=(kt == 0), stop=(kt == KT - 1),
                                     tile_position=(0, p))
                for pj in (2, 1, 0):
                    pq = st2t()
                    for kt in range(KT):
                        nc.tensor.matmul(
                            pq[:, 0:512],
                            wq_sb[kt][:, b * 384 + pj * 128:
                                      b * 384 + (pj + 1) * 128],
                            xg[g][kt][:, sl_g],
                            start=(kt == 0), stop=(kt == KT - 1))
                    if pj == 2:
                        vr = persist.tile([128, 512], bf16, tag="vraw",
                                          bufs=4, name=f"vraw{isl}")
                        vraw[isl] = vr
                        nc.scalar.copy(vr[:], pq[:, 0:512])
                    elif pj == 1:
                        nc.scalar.copy(k2T[:, sl], pq[:, 0:512])
                    else:
                        nc.scalar.copy(q2T[:, sl], pq[:, 0:512])

            def newton_pair(g, rr, sA, sB):
                """rsqrt stats + broadcasts + corrections + V transposes for
                isls g*4+rr, g*4+rr+1 (stat rows 32*rr, 32*(rr+1))."""
                b = g
                tsum = work.tile([128, 512], f32, tag="tsum", bufs=2)
                tsq = work.tile([128, 512], f32, tag="tsq", bufs=2)
                nc.scalar.copy(tsum[:], sA[:])
                nc.scalar.copy(tsq[:], sB[:])
                mean = work.tile([128, 512], f32, tag="nmean", bufs=2)
                nc.vector.tensor_scalar(mean[:], tsum[:], 1.0 / DIM, None, ALU.mult)
                var = work.tile([128, 512], f32, tag="nvar", bufs=2)
                nc.vector.tensor_scalar(var[:], tsq[:], 1.0 / DIM, 1e-5,
                                        ALU.mult, ALU.add)
                msq = work.tile([128, 512], f32, tag="nmsq", bufs=2)
                nc.vector.tensor_tensor(msq[:], mean[:], mean[:], op=ALU.mult)
                nc.vector.tensor_tensor(var[:], var[:], msq[:], op=ALU.subtract)
                u = work.tile([128, 512], f32, tag="nu", bufs=2)
                nc.vector.tensor_scalar(u[:], var[:], -0.5, 1.5, ALU.mult, ALU.add)
                nwt = work.tile([128, 512], f32, tag="nwt", bufs=2)
                for _ in range(2):
                    nc.vector.tensor_tensor(nwt[:], u[:], u[:], op=ALU.mult)
                    nc.vector.tensor_tensor(nwt[:], nwt[:], var[:], op=ALU.mult)
                    nc.vector.tensor_scalar(nwt[:], nwt[:], -0.5, 1.5,
                                            ALU.mult, ALU.add)
                    nc.vector.tensor_tensor(u[:], u[:], nwt[:], op=ALU.mult)
                mu = work.tile([128, 512], f32, tag="nmu", bufs=2)
                nc.vector.tensor_tensor(mu[:], mean[:], u[:], op=ALU.mult)
                ub = work.tile([128, 512], bf16, tag="nub", bufs=2)
                mub = work.tile([128, 512], bf16, tag="nmub", bufs=2)
                nc.vector.tensor_copy(ub[:], u[:])
                nc.vector.tensor_copy(mub[:], mu[:])

                for r in (rr, rr + 1):
                    isl = g * 4 + r
                    sl = slice(isl * 512, (isl + 1) * 512)
                    p = 32 * r
                    pU = st2t()
                    nc.tensor.matmul(pU[:, 0:512], ones_b[p:p + 1, :],
                                     ub[p:p + 1, :],
                                     start=True, stop=True, tile_position=(p, 0))
                    nc.tensor.matmul(pU[:, 512:1024], ones_b[p:p + 1, :],
                                     mub[p:p + 1, :],
                                     start=True, stop=True, tile_position=(p, 0))
                    usb = persist.tile([128, 512], bf16, tag="Usb", bufs=NSL,
                                       name=f"U{isl}")
                    nc.scalar.copy(usb[:], pU[:, 0:512])
                    U_sb[isl] = usb
                    musb = persist.tile([128, 512], bf16, tag="MUsb", bufs=NSL,
                                        name=f"MU{isl}")
                    nc.scalar.copy(musb[:], pU[:, 512:1024])
                    MU_sb[isl] = musb
                    for pj, dest in ((0, q2T[:, sl]), (1, k2T[:, sl]),
                                     (2, vraw[isl][:])):
                        w2 = work.tile([128, 512], bf16, tag="w2", bufs=3)
                        nc.gpsimd.tensor_scalar(w2[:], musb[:],
                                                wc[:, b * 6 + pj:b * 6 + pj + 1],
                                                wc[:, b * 6 + 3 + pj:b * 6 + 4 + pj],
                                                ALU.mult, ALU.add)
                        nc.vector.tensor_tensor(dest, dest, usb[:], op=ALU.mult)
                        nc.vector.tensor_tensor(dest, dest, w2[:], op=ALU.add)
                    for q4 in range(4):
                        gj = b * JT + r * 4 + q4
                        v2 = persist.tile([128, 128], bf16, tag="V2", bufs=B * JT,
                                          name=f"V2_{gj}")
                        tq[gj % 2].dma_start_transpose(
                            v2[:], vraw[isl][:, q4 * 128:(q4 + 1) * 128])
                        V2[gj] = v2

            def qkv_group(g):
                sA = podt()   # LN sums,    isl r at partition 32r
                sB = podt()   # LN sumsqs,  isl r at partition 32r
                qkv_isl(g, 0, sA, sB)
                qkv_isl(g, 1, sA, sB)
                newton_pair(g, 0, sA, sB)
                qkv_isl(g, 2, sA, sB)
                qkv_isl(g, 3, sA, sB)
                newton_pair(g, 2, sA, sB)

            def attn_slice(b, islq, prev_norm, prev_proj):
                isl = b * 4 + islq
                qsl = slice(b * N + islq * 512, b * N + (islq + 1) * 512)
                acc = [None] * 4          # po_A, po_B, dn_A, dn_B
                pt2s = [None] * JT

                def avden(jt):
                    # 8 matmuls, all (64,64) tiles: 4-way concurrent quads.
                    # keylo rows -> *_A banks, keyhi rows -> *_B banks.
                    lst = (jt == JT - 1)
                    gj = b * JT + jt
                    pt2 = pt2s[jt]
                    po_a, po_b, dn_a, dn_b = acc
                    for h in range(2):
                        csl = slice(h * 512, (h + 1) * 512)
                        dsl = slice(h * 64, (h + 1) * 64)
                        nc.tensor.matmul(po_a[dsl, :], V2[gj][0:64, dsl],
                                         pt2[0:64, csl], start=False, stop=lst,
                                         tile_position=(0, h * 64))
                        nc.tensor.matmul(po_b[dsl, :], V2[gj][64:128, dsl],
                                         pt2[64:128, csl], start=False, stop=lst,
                                         tile_position=(64, h * 64))
                        nc.tensor.matmul(dn_a[dsl, :], ones64[0:64, :],
                                         pt2[0:64, csl], start=False, stop=lst,
                                         tile_position=(0, h * 64))
                        nc.tensor.matmul(dn_b[dsl, :], ones64[64:128, :],
                                         pt2[64:128, csl], start=False, stop=lst,
                                         tile_position=(64, h * 64))

                for jt in range(JT):
                    klo = slice(b * N + jt * 128, b * N + jt * 128 + 64)
                    khi = slice(b * N + jt * 128 + 64, b * N + (jt + 1) * 128)
                    st = st2t()
                    for h in range(2):
                        rsl = slice(h * 64, (h + 1) * 64)
                        csl = slice(h * 512, (h + 1) * 512)
                        nc.tensor.matmul(st[0:64, csl], k2T[rsl, klo],
                                         q2T[rsl, qsl], start=True, stop=True,
                                         tile_position=(h * 64, 0))
                        nc.tensor.matmul(st[64:128, csl], k2T[rsl, khi],
                                         q2T[rsl, qsl], start=True, stop=True,
                                         tile_position=(h * 64, 64))
                    pt2 = work.tile([128, 1024], bf16, tag="pt2", bufs=AV_LAG + 2)
                    if jt % 2 == 1:
                        nc.vector.tensor_scalar(pt2[:].bitcast(i16), st[:],
                                                A_SCH, B_SCH, ALU.mult, ALU.add)
                    else:
                        nc.scalar.activation(pt2[:], st[:], AF.Exp, scale=DH ** -0.5)
                    pt2s[jt] = pt2
                    if jt == 2 and prev_norm is not None:
                        prev_norm()
                    if jt == AV_LAG - 1:
                        # zero-init the 4 accumulator banks (robust under either
                        # has_written-clear semantics; attn matmuls accumulate)
                        acc = [podt() for _ in range(4)]
                        for pz in acc:
                            nc.tensor.matmul(pz[0:64, :], zeros64[0:64, :],
                                             q2T[0:64, qsl], start=True,
                                             stop=True, tile_position=(0, 0))
                            nc.tensor.matmul(pz[64:128, :], zeros64[0:64, :],
                                             q2T[0:64, qsl], start=True,
                                             stop=True, tile_position=(0, 64))
                    if jt >= AV_LAG:
                        avden(jt - AV_LAG)
                for jt in range(JT - AV_LAG, JT):
                    avden(jt)
                po_a, po_b, dn_a, dn_b = acc
                o2t = work.tile([128, 512], bf16, tag="o2t", bufs=2)

                def norm():
                    # bank-freeing part runs at high priority so the next
                    # slice's zero-init matmuls get the psum banks quickly;
                    # the slow reciprocal stays at normal priority.
                    dsum = work.tile([128, 512], f32, tag="dsum", bufs=2)
                    osum = work.tile([128, 512], bf16, tag="osum", bufs=2)
                    with tc.high_priority():
                        pob_sb = work.tile([128, 512], bf16, tag="pob", bufs=2)
                        nc.scalar.copy(pob_sb[:], po_b[:])
                        dnb_sb = work.tile([128, 512], f32, tag="dnb", bufs=2)
                        nc.scalar.copy(dnb_sb[:], dn_b[:])
                        nc.vector.tensor_tensor(osum[:], po_a[:], pob_sb[:],
                                                op=ALU.add)
                        nc.vector.tensor_tensor(dsum[:], dn_a[:], dnb_sb[:],
                                                op=ALU.add)
                    rb = work.tile([128, 512], f32, tag="rb", bufs=2)
                    nc.vector.reciprocal(rb[:], dsum[:])
                    nc.gpsimd.tensor_tensor(o2t[:], osum[:], rb[:], op=ALU.mult)

                def proj():
                    for half in range(4):
                        py = st2t()
                        for k in range(2):
                            ncx = half * 2 + k
                            nc.tensor.matmul(py[:, 512 * k:512 * (k + 1)],
                                             wo_sb[:, ncx * 128:(ncx + 1) * 128],
                                             o2t[:], start=True, stop=True)
                        yb = work.tile([128, 1024], bf16, tag="yb", bufs=3)
                        nc.scalar.copy(yb[:], py[:])
                        for k in range(2):
                            ncx = half * 2 + k
                            yq[(half + k) % 2].dma_start(
                                yT_out[ncx * 128:(ncx + 1) * 128, qsl],
                                yb[:, 512 * k:512 * (k + 1)])
                if prev_proj is not None:
                    prev_proj()
                return norm, proj

            qkv_group(0)
            pn = pp = None
            for islq in range(4):
                pn, pp = attn_slice(0, islq, pn, pp)
            pn()
            pp()
            qkv_group(1)
            pn = pp = None
            for islq in range(4):
                pn, pp = attn_slice(1, islq, pn, pp)
            pn()
            pp()

    nc.compile()
    return nc


_NC_CACHE = None


def _get_nc():
    global _NC_CACHE
    if _NC_CACHE is None:
        _NC_CACHE = build_program()
    return _NC_CACHE


def make_in_maps(x, conditioning_embeddings, gamma, cond_W, cond_b, Wq, Wkv, Wo):
    x = np.asarray(x, np.float32)
    ce = np.asarray(conditioning_embeddings, np.float32)
    gamma = np.asarray(gamma, np.float32)
    cond_W = np.asarray(cond_W, np.float32)
    cond_b = np.asarray(cond_b, np.float32)
    Wq = np.asarray(Wq, np.float32)
    Wkv = np.asarray(Wkv, np.float32)
    Wo = np.asarray(Wo, np.float32)

    bf = ml_dtypes.bfloat16
    xT = np.ascontiguousarray(x.reshape(TOK, DIM).T).astype(bf)

    # FiLM on host: silu -> linear -> (scale, shift); fold into QKV weights
    cond = (ce / (1.0 + np.exp(-ce))) @ cond_W + cond_b          # [B, 2D]
    scale, shift = cond[:, :DIM], cond[:, DIM:]                   # [B, D]
    gpf = (scale + 1.0) * gamma                                   # [B, D]

    in_maps = []
    for c in range(NCORES):
        cs = slice(128 * c, 128 * (c + 1))
        Wc = np.concatenate(
            [Wq[:, cs], Wkv[:, cs], Wkv[:, 1024 + 128 * c:1024 + 128 * (c + 1)]],
            axis=1)                                               # [D, 384]
        wq_b = []
        for b in range(B):
            wgb = (Wc * gpf[b][:, None]).astype(bf)               # [D, 384] bf16
            wq_b.append(wgb)
        # wcorr layout: col b*6+p = wgs_neg[b] slice p; col b*6+3+p = wbs[b] slice p
        wcorr = np.zeros((128, 12), np.float32)
        for b in range(B):
            wgs_neg = -wq_b[b].astype(np.float32).sum(axis=0)
            wbs = shift[b] @ Wc
            for p in range(3):
                wcorr[:, b * 6 + p] = wgs_neg[p * 128:(p + 1) * 128]
                wcorr[:, b * 6 + 3 + p] = wbs[p * 128:(p + 1) * 128]
        in_maps.append({
            "xT": xT,
            "wqkv": np.ascontiguousarray(np.concatenate(wq_b, axis=1)),
            "wcorr": wcorr,
            "wo": np.ascontiguousarray(Wo[cs, :]).astype(bf),
        })
    return in_maps


def kernel(**inputs) -> np.ndarray:
    nc = _get_nc()
    in_maps = make_in_maps(**inputs)
    res = run_bass_kernel_spmd(nc, in_maps, core_ids=list(range(NCORES)))
    acc = np.zeros((DIM, TOK), np.float32)
    for core in res.results:
        acc += np.asarray(core["yT"]).astype(np.float32)
    return np.ascontiguousarray(acc.T).reshape(B, N, DIM)
